# revision 23
# baseline (speedup 1.0000x reference)
"""Trainium2 Bass kernel for ragged bag-attention (nn_Attention).

Algorithm (per sentence i, bag b): logit_i = <x_i, att[q_i]*rel[q_i]>;
w = softmax(logit) within bag; bag_repr_b = sum w_i x_i; out = bag_repr @ rel.T + bias.

Transposed device strategy (8 cores, sentence-sharded):
  - Host pre-transposes x into 6 d-blocks of 115 rows so PE contracts over d:
        PZ = xT.T @ [cw.T | rel.T]   -> [128, 106] PSUM   (6 matmuls, 106 moving)
    where cw = att*rel. Cols 0:53 are per-class logits P, cols 53:106 are
    class-space projections Z = x @ rel.T.
  - ACT: ef = exp(P) for all 53 classes (bf16).
  - DVE: e_i = sum_c ef[i,c]*(c==q_i)  (scalar_tensor_tensor w/ accum);
         ET[i,s] = (s==slot_i)*e_i     (tensor_scalar), slots 0:32 per chunk pair.
  - Pool: Z copied PSUM->SBUF bf16.
  - PE: bag numerators in CLASS space: bag[32] += ET.T @ [Z | 1]  (54 moving).
  - Host: bin fragment rows by bag, divide by denominators, add bias.
  Softmax is max-free (|logit| < ~0.6 on this data), so bag fragments from
  different chunks/cores combine by plain summation on host.
"""
import sys
sys.path.insert(0, '/opt/trn_rl_repo')
import numpy as np

NCORES = 8
DIM = 690
NCLS = 53
CHUNK = 128
BSLOT = 16
DB = 6              # d-blocks
DP = 115            # partitions per d-block (6*115 = 690)
SB = 8              # chunks per x superblock (one DMA)
FG = 4              # bag groups (of 8 chunks) per output flush

PAD_SLOT = 99.0     # slot value for padded rows: matches no ET column

XDT = "f8e3"        # x stream dtype: "bf16" or "f8e3" (weights stay bf16)

_cache = {}         # (nchunk, xdt) -> compiled Bass module


def _pack_core(scope, seg, lo, hi):
    """Pack sentences [lo,hi) into chunks of <=CHUNK sentences and <=BSLOT
    bag-fragments. Returns list of chunks, each a list of (bag, start, take)."""
    b0, b1 = int(seg[lo]), int(seg[hi - 1])
    chunks, cur, fill = [], [], 0
    for b in range(b0, b1 + 1):
        s = max(int(scope[b]), lo)
        e = min(int(scope[b + 1]), hi)
        m = e - s
        while m > 0:
            if fill == CHUNK or len(cur) == BSLOT:
                chunks.append(cur)
                cur, fill = [], 0
            take = min(m, CHUNK - fill)
            cur.append((b, s, take))
            fill += take
            s += take
            m -= take
    if cur:
        chunks.append(cur)
    return chunks


def _build_module(nchunk, xdt=XDT):
    from concourse import bacc, mybir
    from concourse.tile import TileContext

    f32 = mybir.dt.float32
    bf16 = mybir.dt.bfloat16
    xmt = mybir.dt.float8e3 if xdt == "f8e3" else bf16
    S = nchunk * CHUNK
    assert nchunk % (SB * FG) == 0
    ngrp = nchunk // SB          # one bag PSUM tile (128 rows) per 8 chunks

    nc = bacc.Bacc()
    xt_d = nc.declare_dram_parameter("xt", [DP, DB, S], xmt, isOutput=False)
    cw_d = nc.declare_dram_parameter("cwrel", [DP, DB * 2 * NCLS], bf16,
                                     isOutput=False)
    qv_d = nc.declare_dram_parameter("qv", [CHUNK, nchunk], f32, isOutput=False)
    rs_d = nc.declare_dram_parameter("rs", [CHUNK, nchunk], f32, isOutput=False)
    io53_d = nc.declare_dram_parameter("io53", [CHUNK, NCLS], bf16, isOutput=False)
    io32_d = nc.declare_dram_parameter("io32", [CHUNK, 2 * BSLOT], bf16,
                                       isOutput=False)
    # per 8-chunk group: [32 slots, 4 pairs * 54] table block
    GW = 4 * (NCLS + 1)
    tab_d = nc.declare_dram_parameter("tab", [ngrp * 32, GW], f32,
                                      isOutput=True)

    W2 = 2 * NCLS    # 106
    DELAY = 2        # bag-matmul groups emitted this many 4-chunk groups late
    with TileContext(nc) as tc:
        with (
            tc.tile_pool(name="consts", bufs=1) as cpool,
            tc.tile_pool(name="xb", bufs=3) as xpool,
            tc.tile_pool(name="ef", bufs=8) as epool,
            tc.tile_pool(name="zs", bufs=2 + DELAY) as zpool,
            tc.tile_pool(name="esel", bufs=6) as espool,
            tc.tile_pool(name="et", bufs=4 * (1 + DELAY)) as etpool,
            tc.tile_pool(name="flush", bufs=2) as fpool,
            tc.tile_pool(name="pz", bufs=4, space="PSUM") as pzpool,
            tc.tile_pool(name="bags", bufs=2, space="PSUM") as bpool,
        ):
            cw_sb = cpool.tile([DP, DB * W2], bf16)
            nc.scalar.dma_start(out=cw_sb[:, :], in_=cw_d[:, :])
            qv_sb = cpool.tile([CHUNK, nchunk], f32)
            nc.scalar.dma_start(out=qv_sb[:, :], in_=qv_d[:, :])
            rs_sb = cpool.tile([CHUNK, nchunk], f32)
            nc.scalar.dma_start(out=rs_sb[:, :], in_=rs_d[:, :])
            io53_sb = cpool.tile([CHUNK, NCLS], bf16)
            nc.scalar.dma_start(out=io53_sb[:, :], in_=io53_d[:, :])
            io32_sb = cpool.tile([CHUNK, 2 * BSLOT], bf16)
            nc.scalar.dma_start(out=io32_sb[:, :], in_=io32_d[:, :])

            state = {"bag": None, "fl": None}

            def emit_bag(h, items):
                # bag matmuls for 4-chunk group h, emitted DELAY groups late
                # so the in-order PE queue never stalls on the ACT/DVE chain.
                sb, h2 = divmod(h, 2)
                if h2 == 0:
                    state["bag"] = bpool.tile([32, GW], f32, name="bag")
                bag = state["bag"]
                for (k, u, m, et, zs4) in items:
                    pc = (u // 2) * (NCLS + 1)
                    first = (k % 2 == 0)
                    nc.tensor.matmul(
                        bag[0:32, pc:pc + NCLS + 1], et[:, :],
                        zs4[:, m * (NCLS + 1):(m + 1) * (NCLS + 1)],
                        start=first, stop=not first)
                if h2 == 1:
                    g = sb % FG
                    if g == 0:
                        state["fl"] = fpool.tile([32, FG * GW], f32, name="fl")
                    fl = state["fl"]
                    nc.scalar.copy(out=fl[:, g * GW:(g + 1) * GW], in_=bag[:, :])
                    if g == FG - 1:
                        q4 = sb // FG
                        dst = tab_d[q4 * FG * 32:(q4 + 1) * FG * 32, :]
                        nc.scalar.dma_start(
                            out=dst.rearrange("(g r) c -> r g c", g=FG),
                            in_=fl[:, :].rearrange("r (g c) -> r g c", g=FG))

            pending = []
            for sb in range(nchunk // SB):
                # one DMA loads 8 chunks of transposed x: [115, 6, 1024]
                xb = xpool.tile([DP, DB * SB * CHUNK], xmt)
                nc.sync.dma_start(
                    out=xb[:, :].rearrange("p (b c) -> p b c", b=DB),
                    in_=xt_d[:, :, sb * SB * CHUNK:(sb + 1) * SB * CHUNK])

                for h2 in range(SB // 4):      # half-superblock: 4 chunks
                    h = sb * 2 + h2
                    pz4 = pzpool.tile([CHUNK, 4 * W2], f32)
                    for m in range(4):
                        u = h2 * 4 + m
                        for b in range(DB):
                            xe = xb[:, (b * SB + u) * CHUNK:
                                    (b * SB + u + 1) * CHUNK]
                            nc.tensor.matmul(pz4[:, m * W2:(m + 1) * W2], xe,
                                             cw_sb[:, b * W2:(b + 1) * W2],
                                             start=(b == 0), stop=(b == DB - 1))

                    # batched PSUM->SBUF: exp of P-half, copy of Z-half.
                    # zs4 blocks are [Z | 1]: col 53 of each 54-block is 1.0
                    # (memset on Pool) so one bag matmul covers numer+denom.
                    ef4 = epool.tile([CHUNK, 4 * NCLS], bf16)
                    nc.scalar.activation(
                        ef4[:, :].rearrange("p (m w) -> p m w", m=4),
                        pz4[:, :].rearrange("p (m w) -> p m w", m=4)[:, :, 0:NCLS],
                        mybir.ActivationFunctionType.Exp)
                    zs4 = zpool.tile([CHUNK, 4 * (NCLS + 1)], bf16)
                    nc.scalar.copy(
                        out=zs4[:, :].rearrange("p (m w) -> p m w", m=4)
                        [:, :, 0:NCLS],
                        in_=pz4[:, :].rearrange("p (m w) -> p m w", m=4)
                        [:, :, NCLS:W2])
                    nc.gpsimd.memset(
                        zs4[:, :].rearrange("p (m w) -> p m w", m=4)
                        [:, :, NCLS:NCLS + 1], 1.0)

                    items = []
                    for m in range(4):
                        u = h2 * 4 + m
                        k = sb * SB + u
                        junk = epool.tile([CHUNK, NCLS], bf16)
                        esel = espool.tile([CHUNK, 1], f32)
                        nc.vector.scalar_tensor_tensor(
                            out=junk[:, :], in0=io53_sb[:, :],
                            scalar=qv_sb[:, k:k + 1],
                            in1=ef4[:, m * NCLS:(m + 1) * NCLS],
                            op0=mybir.AluOpType.is_equal,
                            op1=mybir.AluOpType.mult,
                            accum_out=esel[:, 0:1])

                        et = etpool.tile([CHUNK, 2 * BSLOT], bf16)
                        nc.vector.tensor_scalar(
                            out=et[:, :], in0=io32_sb[:, :],
                            scalar1=rs_sb[:, k:k + 1], scalar2=esel[:, 0:1],
                            op0=mybir.AluOpType.is_equal,
                            op1=mybir.AluOpType.mult)
                        items.append((k, u, m, et, zs4))

                    pending.append((h, items))
                    if len(pending) > DELAY:
                        emit_bag(*pending.pop(0))
            for h, items in pending:
                emit_bag(h, items)

    nc.compile()
    return nc


def _prepare(x, rel_weight, att_weight, bias, attention_query, scope):
    x = np.asarray(x, dtype=np.float32)
    rel_weight = np.asarray(rel_weight, dtype=np.float32)
    att_weight = np.asarray(att_weight, dtype=np.float32)
    bias = np.asarray(bias, dtype=np.float32)
    q = np.asarray(attention_query).astype(np.int64)
    scope = np.asarray(scope).astype(np.int64)

    import ml_dtypes
    bf16 = ml_dtypes.bfloat16
    xnp = ml_dtypes.float8_e3m4 if XDT == "f8e3" else bf16

    nsent = x.shape[0]
    nbags = len(scope) - 1
    score = nsent // NCORES
    seg = (np.searchsorted(scope, np.arange(nsent), side='right') - 1)
    cw = att_weight * rel_weight

    all_chunks = [_pack_core(scope, seg, c * score, (c + 1) * score)
                  for c in range(NCORES)]
    nchunk = max(len(ch) for ch in all_chunks)
    step = SB * FG
    nchunk = (nchunk + step - 1) // step * step
    S = nchunk * CHUNK

    # shared consts
    W6 = np.concatenate([cw, rel_weight], axis=0)          # [106, 690]
    cwrel = np.ascontiguousarray(
        W6.T.reshape(DB, DP, 2 * NCLS).transpose(1, 0, 2)
    ).reshape(DP, DB * 2 * NCLS).astype(bf16)
    io53 = np.ascontiguousarray(
        np.broadcast_to(np.arange(NCLS, dtype=np.float32), (CHUNK, NCLS))
    ).astype(bf16)
    io32 = np.ascontiguousarray(
        np.broadcast_to(np.arange(2 * BSLOT, dtype=np.float32),
                        (CHUNK, 2 * BSLOT))).astype(bf16)
    in_maps = []
    frag2bag = []
    for c in range(NCORES):
        idx = np.full(S, -1, np.int64)
        relseg = np.full(S, PAD_SLOT, np.float32)
        f2b = np.full((nchunk, BSLOT), -1, np.int64)
        for k, ch in enumerate(all_chunks[c]):
            p = k * CHUNK
            for j, (b, s, take) in enumerate(ch):
                idx[p:p + take] = np.arange(s, s + take)
                relseg[p:p + take] = j + BSLOT * (k % 2)
                f2b[k, j] = b
                p += take
        valid = idx >= 0
        xs = np.zeros((S, DIM), np.float32)
        xs[valid] = x[idx[valid]]
        xt = np.ascontiguousarray(
            xs.reshape(S, DB, DP).transpose(2, 1, 0)).astype(xnp)
        qp = np.zeros(S, np.float32)
        qp[valid] = q[idx[valid]].astype(np.float32)
        in_maps.append({
            "xt": xt,
            "cwrel": cwrel,
            "qv": np.ascontiguousarray(qp.reshape(nchunk, CHUNK).T),
            "rs": np.ascontiguousarray(relseg.reshape(nchunk, CHUNK).T),
            "io53": io53,
            "io32": io32,
        })
        frag2bag.append(f2b)
    return in_maps, frag2bag, nchunk, nbags, bias


def _assemble(tables, frag2bag, nchunk, nbags, bias):
    num = np.zeros((nbags, NCLS))
    den = np.zeros(nbags)
    # fragment (k, j): table row (k//8)*32 + slot, col block ((k//2)%4)*54
    # where slot = j + 16*(k%2)
    ks = np.arange(nchunk)[:, None]
    js = np.arange(BSLOT)[None, :]
    slot = js + BSLOT * (ks % 2)
    rows = ((ks // 8) * 32 + slot).ravel()
    cols = (((ks // 2) % 4) * (NCLS + 1) + np.zeros_like(slot)).ravel()
    for c in range(NCORES):
        table = np.asarray(tables[c], dtype=np.float32)
        fb = frag2bag[c].ravel()
        v = fb >= 0
        r = rows[v]
        c0 = cols[v]
        fbv = fb[v]
        for k in range(NCLS):
            num[:, k] += np.bincount(fbv, table[r, c0 + k], minlength=nbags)
        den += np.bincount(fbv, table[r, c0 + NCLS], minlength=nbags)
    return (num / den[:, None] + bias[None, :]).astype(np.float32)


def kernel(x, rel_weight, att_weight, bias, attention_query, scope):
    from concourse.bass_utils import run_bass_kernel_spmd

    in_maps, frag2bag, nchunk, nbags, b = _prepare(
        x, rel_weight, att_weight, bias, attention_query, scope)
    key = (nchunk, XDT)
    if key not in _cache:
        _cache[key] = _build_module(nchunk)
    nc = _cache[key]
    res = run_bass_kernel_spmd(nc, in_maps, list(range(NCORES)))
    tables = [res.results[c]["tab"] for c in range(NCORES)]
    return _assemble(tables, frag2bag, nchunk, nbags, b)


# revision 24
# speedup vs baseline: 1.0841x; 1.0841x over previous
"""Trainium2 Bass kernel for ragged bag-attention (nn_Attention).

Algorithm (per sentence i, bag b): logit_i = <x_i, att[q_i]*rel[q_i]>;
w = softmax(logit) within bag; bag_repr_b = sum w_i x_i; out = bag_repr @ rel.T + bias.

Transposed device strategy (8 cores, sentence-sharded):
  - Host pre-transposes x so PE contracts over d:
        PZ = xT.T @ [cw.T | rel.T]   -> [128, 106] PSUM
    where cw = att*rel. Cols 0:53 are per-class logits P, cols 53:106 are
    class-space projections Z = x @ rel.T.
  - Two sections per core (hybrid precision, split at bag size <= TSMALL):
      big bags:  x, cw, rel in fp8 e4m3 (scaled), DoubleRow matmuls
                 (3 matmuls of K=230 pairs, 0.5 cyc/row);
      small bags: x in fp8 e3m4, weights bf16 (6 matmuls of K=115) --
                 small bags get no quantization averaging, so they need
                 the extra mantissa bits.
  - ACT: ef = exp(P * 1/SC) for all 53 classes (bf16), batched over 4 chunks.
  - DVE: e_i = sum_c ef[i,c]*(c==q_i)  (scalar_tensor_tensor w/ accum);
         ET[i,s] = (s==slot_i)*e_i     (tensor_scalar), slots 0:32 per pair.
  - PE: bag numerators in CLASS space: bag[32] += ET.T @ [Z | 1] (54 moving),
        emitted DELAY groups late so the in-order PE queue never stalls.
  - Host: bin fragment rows by bag, divide by denominators, add bias.
  Softmax is max-free (|logit| < ~0.6 on this data), so bag fragments from
  different chunks/cores combine by plain summation on host.
"""
import sys
sys.path.insert(0, '/opt/trn_rl_repo')
import numpy as np

NCORES = 8
DIM = 690
NCLS = 53
CHUNK = 128
BSLOT = 16
DB = 6              # d-blocks (small section: 6x115; big: 3x(2x115) DoubleRow)
DP = 115            # partitions per d-block (6*115 = 690)
import os
SB = int(os.environ.get("V5_SB", "8"))  # chunks per x superblock (one DMA)
FG = 4              # bag groups (of 8 chunks) per output flush
GW = 4 * (NCLS + 1)
W2 = 2 * NCLS       # 106

PAD_SLOT = 99.0     # slot value for padded rows: matches no ET column
TSMALL = 16         # bags with size <= TSMALL go to the e3m4 section
FP8MAX = 224.0      # safe max for fp8e4 (IEEE e4m3, max normal 240)

_cache = {}         # (nb, ns) -> compiled Bass module


def _schedule(nb, ns):
    """Emission order of 8-chunk superblocks: B spread through the first
    ~3/4 of the run (never at the tail, where its longer ring would drain
    unhidden)."""
    na, nbs = nb // SB, ns // SB
    tot = na + nbs
    lo, hi = min(6, tot // 4), max(1, tot - 7)
    bpos = ({lo + (k * (hi - lo)) // max(1, nbs - 1) for k in range(nbs)}
            if nbs else set())
    out = []
    ia = ib = 0
    for pos in range(tot):
        if pos in bpos and ib < nbs:
            out.append(("B", ib)); ib += 1
        elif ia < na:
            out.append(("A", ia)); ia += 1
        else:
            out.append(("B", ib)); ib += 1
    return out


def _pack_core(scope, seg, lo, hi, keep):
    """Pack sentences of bags with keep[bag]=True in [lo,hi) into chunks of
    <=CHUNK sentences / <=BSLOT fragments. Returns [[(bag, start, take)...]]."""
    b0, b1 = int(seg[lo]), int(seg[hi - 1])
    chunks, cur, fill = [], [], 0
    for b in range(b0, b1 + 1):
        if not keep[b]:
            continue
        s = max(int(scope[b]), lo)
        e = min(int(scope[b + 1]), hi)
        m = e - s
        while m > 0:
            if fill == CHUNK or len(cur) == BSLOT:
                chunks.append(cur)
                cur, fill = [], 0
            take = min(m, CHUNK - fill)
            cur.append((b, s, take))
            fill += take
            s += take
            m -= take
    if cur:
        chunks.append(cur)
    return chunks


def _build_module(nb, ns, DMAQ='alt', DELAY_=2):
    from concourse import bacc, mybir
    from concourse.tile import TileContext

    f32 = mybir.dt.float32
    bf16 = mybir.dt.bfloat16
    f8e3 = mybir.dt.float8e3
    f8e4 = mybir.dt.float8e4
    DR = mybir.MatmulPerfMode.DoubleRow
    assert nb % SB == 0 and ns % SB == 0
    Sb, Ss = nb * CHUNK, ns * CHUNK
    ngrp = (nb + ns) // 8
    G2 = ngrp * 2                    # total 4-chunk groups

    nc = bacc.Bacc()
    xtb_d = nc.declare_dram_parameter("xtb", [DP, 3, 2 * Sb], f8e4,
                                      isOutput=False)
    cwb_d = nc.declare_dram_parameter("cwb", [DP, 3 * 2 * W2], f8e4,
                                      isOutput=False)
    xts_d = nc.declare_dram_parameter("xts", [DP, DB, Ss], f8e3, isOutput=False)
    cws_d = nc.declare_dram_parameter("cws", [DP, DB * W2], bf16, isOutput=False)
    qv_d = nc.declare_dram_parameter("qv", [CHUNK, nb + ns], f32, isOutput=False)
    rs_d = nc.declare_dram_parameter("rs", [CHUNK, nb + ns], f32, isOutput=False)
    io53_d = nc.declare_dram_parameter("io53", [CHUNK, NCLS], bf16, isOutput=False)
    io32_d = nc.declare_dram_parameter("io32", [CHUNK, 2 * BSLOT], bf16,
                                       isOutput=False)
    sc_d = nc.declare_dram_parameter("sc", [CHUNK, 1], f32, isOutput=False)
    tab_d = nc.declare_dram_parameter("tab", [ngrp * 32, GW], f32, isOutput=True)

    DELAY = DELAY_   # bag-matmul groups emitted this many 4-chunk groups late
    with TileContext(nc) as tc:
        with (
            tc.tile_pool(name="consts", bufs=1) as cpool,
            tc.tile_pool(name="xb", bufs=5) as xpool,
            tc.tile_pool(name="ef", bufs=5) as epool,
            tc.tile_pool(name="junk", bufs=10) as jpool,
            tc.tile_pool(name="zs", bufs=2 + DELAY) as zpool,
            tc.tile_pool(name="esel", bufs=12) as espool,
            tc.tile_pool(name="et", bufs=8 * (1 + DELAY)) as etpool,
            tc.tile_pool(name="flush", bufs=2) as fpool,
            tc.tile_pool(name="pz", bufs=3, space="PSUM") as pzpool,
            tc.tile_pool(name="bags", bufs=2, space="PSUM") as bpool,
        ):
            cwb_sb = cpool.tile([DP, 3 * 2 * W2], f8e4)
            nc.scalar.dma_start(out=cwb_sb[:, :], in_=cwb_d[:, :])
            cws_sb = cpool.tile([DP, DB * W2], bf16)
            nc.scalar.dma_start(out=cws_sb[:, :], in_=cws_d[:, :])
            qv_sb = cpool.tile([CHUNK, nb + ns], f32)
            nc.scalar.dma_start(out=qv_sb[:, :], in_=qv_d[:, :])
            rs_sb = cpool.tile([CHUNK, nb + ns], f32)
            nc.scalar.dma_start(out=rs_sb[:, :], in_=rs_d[:, :])
            io53_sb = cpool.tile([CHUNK, NCLS], bf16)
            nc.scalar.dma_start(out=io53_sb[:, :], in_=io53_d[:, :])
            io32_sb = cpool.tile([CHUNK, 2 * BSLOT], bf16)
            nc.scalar.dma_start(out=io32_sb[:, :], in_=io32_d[:, :])
            sc_sb = cpool.tile([CHUNK, 1], f32)
            nc.scalar.dma_start(out=sc_sb[:, :], in_=sc_d[:, :])

            state = {"bag": None, "fl": None}

            def emit_bag(gg, items):
                # bag matmuls for superblock gg, emitted DELAY superblocks
                # late so the in-order PE queue never stalls on the ACT/DVE
                # chain.
                bag = bpool.tile([32, GW], f32, name="bag")
                for (k, m, et, zs8) in items:
                    pc = ((k % 8) // 2) * (NCLS + 1)
                    first = (k % 2 == 0)
                    nc.tensor.matmul(
                        bag[0:32, pc:pc + NCLS + 1], et[:, :],
                        zs8[:, m * (NCLS + 1):(m + 1) * (NCLS + 1)],
                        start=first, stop=not first)
                g = gg % FG
                if g == 0:
                    state["fl"] = fpool.tile([32, FG * GW], f32, name="fl")
                fl = state["fl"]
                nc.scalar.copy(out=fl[:, g * GW:(g + 1) * GW], in_=bag[:, :])
                if g == FG - 1 or gg == ngrp - 1:
                    gu = g + 1
                    q4 = gg // FG
                    dst = tab_d[q4 * FG * 32:q4 * FG * 32 + gu * 32, :]
                    nc.scalar.dma_start(
                        out=dst.rearrange("(g r) c -> r g c", g=gu),
                        in_=fl[:, 0:gu * GW]
                        .rearrange("r (g c) -> r g c", g=gu))

            pending = []

            def group_tail(pos, pz8, is_big):
                # batched PSUM->SBUF over the whole 8-chunk superblock:
                # exp of P-halves, copy of Z-halves. zs8 blocks are [Z | 1]:
                # col 53 of each 54-block is 1.0 (memset on Pool) so one bag
                # matmul covers numer+denom.
                ef8 = epool.tile([CHUNK, SB * NCLS], bf16, name="ef8")
                nc.scalar.activation(
                    ef8[:, :].rearrange("p (m w) -> p m w", m=SB),
                    pz8[:, :].rearrange("p (m w) -> p m w", m=SB)[:, :, 0:NCLS],
                    mybir.ActivationFunctionType.Exp,
                    scale=sc_sb[:, 0:1] if is_big else 1.0)
                zs8 = zpool.tile([CHUNK, SB * (NCLS + 1)], bf16, name="zs8")
                nc.scalar.copy(
                    out=zs8[:, :].rearrange("p (m w) -> p m w", m=SB)
                    [:, :, 0:NCLS],
                    in_=pz8[:, :].rearrange("p (m w) -> p m w", m=SB)
                    [:, :, NCLS:W2])
                nc.gpsimd.memset(
                    zs8[:, :].rearrange("p (m w) -> p m w", m=SB)
                    [:, :, NCLS:NCLS + 1], 1.0)

                items = []
                for m in range(SB):
                    k = pos * SB + m
                    junk = jpool.tile([CHUNK, NCLS], bf16, name="junk")
                    esel = espool.tile([CHUNK, 1], f32, name="esel")
                    nc.vector.scalar_tensor_tensor(
                        out=junk[:, :], in0=io53_sb[:, :],
                        scalar=qv_sb[:, k:k + 1],
                        in1=ef8[:, m * NCLS:(m + 1) * NCLS],
                        op0=mybir.AluOpType.is_equal,
                        op1=mybir.AluOpType.mult,
                        accum_out=esel[:, 0:1])
                    et = etpool.tile([CHUNK, 2 * BSLOT], bf16, name="et")
                    nc.vector.tensor_scalar(
                        out=et[:, :], in0=io32_sb[:, :],
                        scalar1=rs_sb[:, k:k + 1], scalar2=esel[:, 0:1],
                        op0=mybir.AluOpType.is_equal,
                        op1=mybir.AluOpType.mult)
                    items.append((k, m, et, zs8))
                pending.append((pos, items))
                if len(pending) > DELAY:
                    emit_bag(*pending.pop(0))

            # interleaved schedule: B superblocks spread among A's so the
            # small-bag section's longer PE/ACT/DVE ring hides in A's slack
            for pos, (sec, lsb) in enumerate(_schedule(nb, ns)):
                eng = nc.sync if (DMAQ == "sp" or pos % 2 == 0) else nc.gpsimd
                if sec == "A":
                    xb = xpool.tile([DP, 3 * SB * 2 * CHUNK], f8e4, name="xbb")
                    eng.dma_start(
                        out=xb[:, :].rearrange("p (b c) -> p b c", b=3),
                        in_=xtb_d[:, :, lsb * SB * 2 * CHUNK:
                                  (lsb + 1) * SB * 2 * CHUNK])
                else:
                    xb = xpool.tile([DP, DB * SB * CHUNK], f8e3, name="xbs")
                    eng.dma_start(
                        out=xb[:, :].rearrange("p (b c) -> p b c", b=DB),
                        in_=xts_d[:, :, lsb * SB * CHUNK:(lsb + 1) * SB * CHUNK])
                pz8 = pzpool.tile([CHUNK, SB * CHUNK], f32, name="pz8")
                for u in range(SB):
                    o = u * CHUNK
                    if sec == "A":
                        for blk in range(3):
                            sl = xb[:, (blk * SB + u) * 2 * CHUNK:
                                    (blk * SB + u + 1) * 2 * CHUNK]
                            nc.tensor.matmul(
                                pz8[:, o:o + W2],
                                sl.rearrange("k (j m2) -> k j m2", j=2),
                                cwb_sb[:, blk * 2 * W2:(blk + 1) * 2 * W2]
                                .rearrange("k (j n) -> k j n", j=2),
                                start=(blk == 0), stop=(blk == 2),
                                perf_mode=DR)
                    else:
                        for b in range(DB):
                            xe = xb[:, (b * SB + u) * CHUNK:
                                    (b * SB + u + 1) * CHUNK]
                            nc.tensor.matmul(
                                pz8[:, o:o + W2], xe,
                                cws_sb[:, b * W2:(b + 1) * W2],
                                start=(b == 0), stop=(b == DB - 1))
                group_tail(pos, pz8, sec == "A")

            for h, items in pending:
                emit_bag(h, items)

    nc.compile()
    return nc


def _prepare(x, rel_weight, att_weight, bias, attention_query, scope):
    x = np.asarray(x, dtype=np.float32)
    rel_weight = np.asarray(rel_weight, dtype=np.float32)
    att_weight = np.asarray(att_weight, dtype=np.float32)
    bias = np.asarray(bias, dtype=np.float32)
    q = np.asarray(attention_query).astype(np.int64)
    scope = np.asarray(scope).astype(np.int64)

    import ml_dtypes
    bf16 = ml_dtypes.bfloat16
    f8e3 = ml_dtypes.float8_e3m4
    f8e4 = ml_dtypes.float8_e4m3

    nsent = x.shape[0]
    nbags = len(scope) - 1
    score = nsent // NCORES
    seg = (np.searchsorted(scope, np.arange(nsent), side='right') - 1)
    cw = att_weight * rel_weight
    sizes = np.diff(scope)
    isbig = sizes > TSMALL

    SC = 2.0 ** int(np.floor(np.log2(FP8MAX / np.abs(cw).max())))
    SR = 2.0 ** int(np.floor(np.log2(FP8MAX / np.abs(rel_weight).max())))

    big_chunks, small_chunks = [], []
    for c in range(NCORES):
        lo, hi = c * score, (c + 1) * score
        big_chunks.append(_pack_core(scope, seg, lo, hi, isbig))
        small_chunks.append(_pack_core(scope, seg, lo, hi, ~isbig))
    nb = max(len(ch) for ch in big_chunks)
    ns = max(len(ch) for ch in small_chunks)
    nb = (nb + SB - 1) // SB * SB
    ns = max(SB, (ns + SB - 1) // SB * SB)

    # shared consts. cwrel = [cw.T | rel.T] stacked per d-block.
    W6 = np.concatenate([cw, rel_weight], axis=0)          # [106, 690]
    # small: d = b*115 + p, b in 0..5
    cws = np.ascontiguousarray(
        W6.T.reshape(DB, DP, W2).transpose(1, 0, 2)
    ).reshape(DP, DB * W2).astype(bf16)
    # big (DoubleRow): d = blk*230 + j*115 + p, scaled
    W6s = np.concatenate([cw * SC, rel_weight * SR], axis=0)
    cwb = np.ascontiguousarray(
        W6s.T.reshape(3, 2, DP, W2).transpose(2, 0, 1, 3)
    ).reshape(DP, 3 * 2 * W2).astype(f8e4)
    io53 = np.ascontiguousarray(
        np.broadcast_to(np.arange(NCLS, dtype=np.float32), (CHUNK, NCLS))
    ).astype(bf16)
    io32 = np.ascontiguousarray(
        np.broadcast_to(np.arange(2 * BSLOT, dtype=np.float32),
                        (CHUNK, 2 * BSLOT))).astype(bf16)
    scv = np.full((CHUNK, 1), 1.0 / SC, np.float32)

    sched = _schedule(nb, ns)
    # global chunk k = pos*SB + u maps to section-local chunk lsb*SB + u
    gofA = np.zeros(nb // SB, np.int64)
    gofB = np.zeros(ns // SB, np.int64)
    for pos, (sec, lsb) in enumerate(sched):
        (gofA if sec == "A" else gofB)[lsb] = pos

    in_maps = []
    frag2bag = []
    for c in range(NCORES):
        # section-local sentence index arrays (for x packing)
        idxA = np.full(nb * CHUNK, -1, np.int64)
        idxB = np.full(ns * CHUNK, -1, np.int64)
        relseg = np.full((nb + ns) * CHUNK, PAD_SLOT, np.float32)
        qp = np.zeros((nb + ns) * CHUNK, np.float32)
        f2b = np.full((nb + ns, BSLOT), -1, np.int64)
        for sec, chs, idxL, gof in (("A", big_chunks[c], idxA, gofA),
                                    ("B", small_chunks[c], idxB, gofB)):
            for kk, ch in enumerate(chs):
                kglob = gof[kk // SB] * SB + kk % SB
                p = kk * CHUNK
                pg = kglob * CHUNK
                for j, (b, s, take) in enumerate(ch):
                    idxL[p:p + take] = np.arange(s, s + take)
                    relseg[pg:pg + take] = j + BSLOT * (kglob % 2)
                    f2b[kglob, j] = b
                    p += take
                    pg += take
                v = idxL[kk * CHUNK:(kk + 1) * CHUNK] >= 0
                qp[kglob * CHUNK:(kglob + 1) * CHUNK][v] = (
                    q[idxL[kk * CHUNK:(kk + 1) * CHUNK][v]].astype(np.float32))
        # big section x: [S_b, 690] -> [115, 3, S_b*2] DoubleRow interleave
        xsb = np.zeros((nb * CHUNK, DIM), np.float32)
        vb = idxA >= 0
        xsb[vb] = x[idxA[vb]]
        xtb = np.ascontiguousarray(
            xsb.reshape(nb, CHUNK, 3, 2, DP).transpose(4, 2, 0, 3, 1)
        ).reshape(DP, 3, nb * 2 * CHUNK).astype(f8e4)
        # small section x: [S_s, 690] -> [115, 6, S_s]
        xss = np.zeros((ns * CHUNK, DIM), np.float32)
        vs = idxB >= 0
        xss[vs] = x[idxB[vs]]
        xts = np.ascontiguousarray(
            xss.reshape(ns * CHUNK, DB, DP).transpose(2, 1, 0)).astype(f8e3)
        in_maps.append({
            "xtb": xtb,
            "xts": xts,
            "cwb": cwb,
            "cws": cws,
            "qv": np.ascontiguousarray(qp.reshape(nb + ns, CHUNK).T),
            "rs": np.ascontiguousarray(relseg.reshape(nb + ns, CHUNK).T),
            "io53": io53,
            "io32": io32,
            "sc": scv,
        })
        frag2bag.append(f2b)
    return in_maps, frag2bag, nb, ns, nbags, bias, SR


def _assemble(tables, frag2bag, nb, ns, nbags, bias, SR):
    num = np.zeros((nbags, NCLS))
    den = np.zeros(nbags)
    # fragment (k, j): table row (k//8)*32 + slot, col block ((k//2)%4)*54
    # where slot = j + 16*(k%2). Big-section numerators carry factor SR.
    ks = np.arange(nb + ns)[:, None]
    js = np.arange(BSLOT)[None, :]
    slot = js + BSLOT * (ks % 2)
    rows = ((ks // 8) * 32 + slot).ravel()
    cols = (((ks // 2) % 4) * (NCLS + 1) + np.zeros_like(slot)).ravel()
    isbigk = np.zeros(nb + ns, bool)
    for pos, (sec, lsb) in enumerate(_schedule(nb, ns)):
        if sec == "A":
            isbigk[pos * SB:(pos + 1) * SB] = True
    nsc = np.where(isbigk[:, None], 1.0 / SR, 1.0).repeat(BSLOT, 1).ravel()
    for c in range(NCORES):
        table = np.asarray(tables[c], dtype=np.float32)
        fb = frag2bag[c].ravel()
        v = fb >= 0
        r = rows[v]
        c0 = cols[v]
        fbv = fb[v]
        sc = nsc[v]
        for k in range(NCLS):
            num[:, k] += np.bincount(fbv, table[r, c0 + k] * sc,
                                     minlength=nbags)
        den += np.bincount(fbv, table[r, c0 + NCLS], minlength=nbags)
    return (num / den[:, None] + bias[None, :]).astype(np.float32)


def kernel(x, rel_weight, att_weight, bias, attention_query, scope):
    from concourse.bass_utils import run_bass_kernel_spmd

    in_maps, frag2bag, nb, ns, nbags, b, SR = _prepare(
        x, rel_weight, att_weight, bias, attention_query, scope)
    key = (nb, ns)
    if key not in _cache:
        _cache[key] = _build_module(nb, ns)
    nc = _cache[key]
    res = run_bass_kernel_spmd(nc, in_maps, list(range(NCORES)))
    tables = [res.results[c]["tab"] for c in range(NCORES)]
    return _assemble(tables, frag2bag, nb, ns, nbags, b, SR)


# revision 25
# speedup vs baseline: 1.0944x; 1.0095x over previous
"""Trainium2 Bass kernel for ragged bag-attention (nn_Attention).

Algorithm (per sentence i, bag b): logit_i = <x_i, att[q_i]*rel[q_i]>;
w = softmax(logit) within bag; bag_repr_b = sum w_i x_i; out = bag_repr @ rel.T + bias.

Transposed device strategy (8 cores, sentence-sharded):
  - Host pre-transposes x so PE contracts over d:
        PZ = xT.T @ [cw.T | rel.T]   -> [128, 106] PSUM
    where cw = att*rel. Cols 0:53 are per-class logits P, cols 53:106 are
    class-space projections Z = x @ rel.T.
  - Two sections per core (hybrid precision, split at bag size <= TSMALL):
      big bags:  x, cw, rel in fp8 e4m3 (scaled), DoubleRow matmuls
                 (3 matmuls of K=230 pairs, 0.5 cyc/row);
      small bags: x in fp8 e3m4, weights bf16 (6 matmuls of K=115) --
                 small bags get no quantization averaging, so they need
                 the extra mantissa bits.
  - ACT: ef = exp(P * 1/SC) for all 53 classes (bf16), batched over 4 chunks.
  - DVE: e_i = sum_c ef[i,c]*(c==q_i)  (scalar_tensor_tensor w/ accum);
         ET[i,s] = (s==slot_i)*e_i     (tensor_scalar), slots 0:32 per pair.
  - PE: bag numerators in CLASS space: bag[32] += ET.T @ [Z | 1] (54 moving),
        emitted DELAY groups late so the in-order PE queue never stalls.
  - Host: bin fragment rows by bag, divide by denominators, add bias.
  Softmax is max-free (|logit| < ~0.6 on this data), so bag fragments from
  different chunks/cores combine by plain summation on host.
"""
import sys
sys.path.insert(0, '/opt/trn_rl_repo')
import numpy as np

NCORES = 8
DIM = 690
NCLS = 53
CHUNK = 128
BSLOT = 16
DB = 6              # d-blocks (small section: 6x115; big: 3x(2x115) DoubleRow)
DP = 115            # partitions per d-block (6*115 = 690)
import os
SB = int(os.environ.get("V5_SB", "8"))  # chunks per x superblock (one DMA)
FG = 4              # bag groups (of 8 chunks) per output flush
GW = 4 * (NCLS + 1)
W2 = 2 * NCLS       # 106

PAD_SLOT = 99.0     # slot value for padded rows: matches no ET column
TSMALL = 16         # bags with size <= TSMALL go to the e3m4 section
FP8MAX = 224.0      # safe max for fp8e4 (IEEE e4m3, max normal 240)

_cache = {}         # (nb, ns) -> compiled Bass module


def _schedule(nb, ns):
    """Emission order of 8-chunk superblocks: B spread through the first
    ~3/4 of the run (never at the tail, where its longer ring would drain
    unhidden)."""
    na, nbs = nb // SB, ns // SB
    tot = na + nbs
    lo, hi = min(6, tot // 4), max(1, tot - 7)
    bpos = ({lo + (k * (hi - lo)) // max(1, nbs - 1) for k in range(nbs)}
            if nbs else set())
    out = []
    ia = ib = 0
    for pos in range(tot):
        if pos in bpos and ib < nbs:
            out.append(("B", ib)); ib += 1
        elif ia < na:
            out.append(("A", ia)); ia += 1
        else:
            out.append(("B", ib)); ib += 1
    return out


def _pack_core(scope, seg, lo, hi, keep):
    """Pack sentences of bags with keep[bag]=True in [lo,hi) into chunks of
    <=CHUNK sentences / <=BSLOT fragments. Returns [[(bag, start, take)...]]."""
    b0, b1 = int(seg[lo]), int(seg[hi - 1])
    chunks, cur, fill = [], [], 0
    for b in range(b0, b1 + 1):
        if not keep[b]:
            continue
        s = max(int(scope[b]), lo)
        e = min(int(scope[b + 1]), hi)
        m = e - s
        while m > 0:
            if fill == CHUNK or len(cur) == BSLOT:
                chunks.append(cur)
                cur, fill = [], 0
            take = min(m, CHUNK - fill)
            cur.append((b, s, take))
            fill += take
            s += take
            m -= take
    if cur:
        chunks.append(cur)
    return chunks


def _build_module(nb, ns, DMAQ='alt', DELAY_=2):
    from concourse import bacc, mybir
    from concourse.tile import TileContext

    f32 = mybir.dt.float32
    bf16 = mybir.dt.bfloat16
    f8e3 = mybir.dt.float8e3
    f8e4 = mybir.dt.float8e4
    DR = mybir.MatmulPerfMode.DoubleRow
    assert nb % SB == 0 and ns % SB == 0
    Sb, Ss = nb * CHUNK, ns * CHUNK
    ngrp = (nb + ns) // 8
    G2 = ngrp * 2                    # total 4-chunk groups

    nc = bacc.Bacc()
    xtb_d = nc.declare_dram_parameter("xtb", [DP, 3, 2 * Sb], f8e4,
                                      isOutput=False)
    cwb_d = nc.declare_dram_parameter("cwb", [DP, 3 * 2 * W2], f8e4,
                                      isOutput=False)
    xts_d = nc.declare_dram_parameter("xts", [DP, DB, Ss], f8e3, isOutput=False)
    cws_d = nc.declare_dram_parameter("cws", [DP, DB * W2], bf16, isOutput=False)
    qv_d = nc.declare_dram_parameter("qv", [CHUNK, nb + ns], f32, isOutput=False)
    rs_d = nc.declare_dram_parameter("rs", [CHUNK, nb + ns], f32, isOutput=False)
    io53_d = nc.declare_dram_parameter("io53", [CHUNK, NCLS], bf16, isOutput=False)
    io32_d = nc.declare_dram_parameter("io32", [CHUNK, 2 * BSLOT], bf16,
                                       isOutput=False)
    sc_d = nc.declare_dram_parameter("sc", [CHUNK, 1], f32, isOutput=False)
    tab_d = nc.declare_dram_parameter("tab", [ngrp * 32, GW], f32, isOutput=True)

    DELAY = DELAY_   # bag-matmul groups emitted this many 4-chunk groups late
    with TileContext(nc) as tc:
        with (
            tc.tile_pool(name="consts", bufs=1) as cpool,
            tc.tile_pool(name="xb", bufs=5) as xpool,
            tc.tile_pool(name="ef", bufs=5) as epool,
            tc.tile_pool(name="junk", bufs=10) as jpool,
            tc.tile_pool(name="zs", bufs=2 + DELAY) as zpool,
            tc.tile_pool(name="esel", bufs=12) as espool,
            tc.tile_pool(name="et", bufs=8 * (1 + DELAY)) as etpool,
            tc.tile_pool(name="flush", bufs=2) as fpool,
            tc.tile_pool(name="pz", bufs=3, space="PSUM") as pzpool,
            tc.tile_pool(name="bags", bufs=2, space="PSUM") as bpool,
        ):
            cwb_sb = cpool.tile([DP, 3 * 2 * W2], f8e4)
            nc.scalar.dma_start(out=cwb_sb[:, :], in_=cwb_d[:, :])
            cws_sb = cpool.tile([DP, DB * W2], bf16)
            nc.scalar.dma_start(out=cws_sb[:, :], in_=cws_d[:, :])
            qv_sb = cpool.tile([CHUNK, nb + ns], f32)
            nc.scalar.dma_start(out=qv_sb[:, :], in_=qv_d[:, :])
            rs_sb = cpool.tile([CHUNK, nb + ns], f32)
            nc.scalar.dma_start(out=rs_sb[:, :], in_=rs_d[:, :])
            io53_sb = cpool.tile([CHUNK, NCLS], bf16)
            nc.scalar.dma_start(out=io53_sb[:, :], in_=io53_d[:, :])
            io32_sb = cpool.tile([CHUNK, 2 * BSLOT], bf16)
            nc.scalar.dma_start(out=io32_sb[:, :], in_=io32_d[:, :])
            sc_sb = cpool.tile([CHUNK, 1], f32)
            nc.scalar.dma_start(out=sc_sb[:, :], in_=sc_d[:, :])

            state = {"bag": None, "fl": None}

            def emit_bag(gg, items):
                # bag matmuls for superblock gg, emitted DELAY superblocks
                # late so the in-order PE queue never stalls on the ACT/DVE
                # chain.
                bag = bpool.tile([32, GW], f32, name="bag")
                for (k, m, et, zs8) in items:
                    pc = ((k % 8) // 2) * (NCLS + 1)
                    first = (k % 2 == 0)
                    nc.tensor.matmul(
                        bag[0:32, pc:pc + NCLS + 1], et[:, :],
                        zs8[:, m * (NCLS + 1):(m + 1) * (NCLS + 1)],
                        start=first, stop=not first)
                g = gg % FG
                if g == 0:
                    state["fl"] = fpool.tile([32, FG * GW], f32, name="fl")
                fl = state["fl"]
                nc.scalar.copy(out=fl[:, g * GW:(g + 1) * GW], in_=bag[:, :])
                if g == FG - 1 or gg == ngrp - 1:
                    gu = g + 1
                    q4 = gg // FG
                    dst = tab_d[q4 * FG * 32:q4 * FG * 32 + gu * 32, :]
                    nc.scalar.dma_start(
                        out=dst.rearrange("(g r) c -> r g c", g=gu),
                        in_=fl[:, 0:gu * GW]
                        .rearrange("r (g c) -> r g c", g=gu))

            pending = []

            def group_tail(pos, pz8, is_big):
                # batched PSUM->SBUF over the whole 8-chunk superblock:
                # exp of P-halves, copy of Z-halves. zs8 blocks are [Z | 1]:
                # col 53 of each 54-block is 1.0 (memset on Pool) so one bag
                # matmul covers numer+denom.
                ef8 = epool.tile([CHUNK, SB * NCLS], bf16, name="ef8")
                nc.scalar.activation(
                    ef8[:, :].rearrange("p (m w) -> p m w", m=SB),
                    pz8[:, :].rearrange("p (m w) -> p m w", m=SB)[:, :, 0:NCLS],
                    mybir.ActivationFunctionType.Exp,
                    scale=sc_sb[:, 0:1] if is_big else 1.0)
                zs8 = zpool.tile([CHUNK, SB * (NCLS + 1)], bf16, name="zs8")
                nc.scalar.copy(
                    out=zs8[:, :].rearrange("p (m w) -> p m w", m=SB)
                    [:, :, 0:NCLS],
                    in_=pz8[:, :].rearrange("p (m w) -> p m w", m=SB)
                    [:, :, NCLS:W2])
                nc.gpsimd.memset(
                    zs8[:, :].rearrange("p (m w) -> p m w", m=SB)
                    [:, :, NCLS:NCLS + 1], 1.0)

                items = []
                for m in range(SB):
                    k = pos * SB + m
                    junk = jpool.tile([CHUNK, NCLS], bf16, name="junk")
                    esel = espool.tile([CHUNK, 1], f32, name="esel")
                    nc.vector.scalar_tensor_tensor(
                        out=junk[:, :], in0=io53_sb[:, :],
                        scalar=qv_sb[:, k:k + 1],
                        in1=ef8[:, m * NCLS:(m + 1) * NCLS],
                        op0=mybir.AluOpType.is_equal,
                        op1=mybir.AluOpType.mult,
                        accum_out=esel[:, 0:1])
                    et = etpool.tile([CHUNK, 2 * BSLOT], bf16, name="et")
                    nc.vector.tensor_scalar(
                        out=et[:, :], in0=io32_sb[:, :],
                        scalar1=rs_sb[:, k:k + 1], scalar2=esel[:, 0:1],
                        op0=mybir.AluOpType.is_equal,
                        op1=mybir.AluOpType.mult)
                    items.append((k, m, et, zs8))
                pending.append((pos, items))
                if len(pending) > DELAY:
                    emit_bag(*pending.pop(0))

            # interleaved schedule: B superblocks spread among A's so the
            # small-bag section's longer PE/ACT/DVE ring hides in A's slack
            for pos, (sec, lsb) in enumerate(_schedule(nb, ns)):
                if sec == "A":
                    xb = xpool.tile([DP, 3 * SB * 2 * CHUNK], f8e4, name="xbb")
                    W = SB * 2 * CHUNK
                    for hf in range(2):
                        eng = (nc.sync if (DMAQ == "sp" or (pos * 2 + hf) % 2 == 0)
                               else nc.gpsimd)
                        eng.dma_start(
                            out=xb[:, :].rearrange("p (b c) -> p b c", b=3)
                            [:, :, hf * (W // 2):(hf + 1) * (W // 2)],
                            in_=xtb_d[:, :, lsb * W + hf * (W // 2):
                                      lsb * W + (hf + 1) * (W // 2)])
                else:
                    xb = xpool.tile([DP, DB * SB * CHUNK], f8e3, name="xbs")
                    W = SB * CHUNK
                    for hf in range(2):
                        eng = (nc.sync if (DMAQ == "sp" or (pos * 2 + hf) % 2 == 0)
                               else nc.gpsimd)
                        eng.dma_start(
                            out=xb[:, :].rearrange("p (b c) -> p b c", b=DB)
                            [:, :, hf * (W // 2):(hf + 1) * (W // 2)],
                            in_=xts_d[:, :, lsb * W + hf * (W // 2):
                                      lsb * W + (hf + 1) * (W // 2)])
                pz8 = pzpool.tile([CHUNK, SB * CHUNK], f32, name="pz8")
                for u in range(SB):
                    o = u * CHUNK
                    if sec == "A":
                        for blk in range(3):
                            sl = xb[:, (blk * SB + u) * 2 * CHUNK:
                                    (blk * SB + u + 1) * 2 * CHUNK]
                            nc.tensor.matmul(
                                pz8[:, o:o + W2],
                                sl.rearrange("k (j m2) -> k j m2", j=2),
                                cwb_sb[:, blk * 2 * W2:(blk + 1) * 2 * W2]
                                .rearrange("k (j n) -> k j n", j=2),
                                start=(blk == 0), stop=(blk == 2),
                                perf_mode=DR)
                    else:
                        for b in range(DB):
                            xe = xb[:, (b * SB + u) * CHUNK:
                                    (b * SB + u + 1) * CHUNK]
                            nc.tensor.matmul(
                                pz8[:, o:o + W2], xe,
                                cws_sb[:, b * W2:(b + 1) * W2],
                                start=(b == 0), stop=(b == DB - 1))
                group_tail(pos, pz8, sec == "A")

            for h, items in pending:
                emit_bag(h, items)

    nc.compile()
    return nc


def _prepare(x, rel_weight, att_weight, bias, attention_query, scope):
    x = np.asarray(x, dtype=np.float32)
    rel_weight = np.asarray(rel_weight, dtype=np.float32)
    att_weight = np.asarray(att_weight, dtype=np.float32)
    bias = np.asarray(bias, dtype=np.float32)
    q = np.asarray(attention_query).astype(np.int64)
    scope = np.asarray(scope).astype(np.int64)

    import ml_dtypes
    bf16 = ml_dtypes.bfloat16
    f8e3 = ml_dtypes.float8_e3m4
    f8e4 = ml_dtypes.float8_e4m3

    nsent = x.shape[0]
    nbags = len(scope) - 1
    score = nsent // NCORES
    seg = (np.searchsorted(scope, np.arange(nsent), side='right') - 1)
    cw = att_weight * rel_weight
    sizes = np.diff(scope)
    isbig = sizes > TSMALL

    SC = 2.0 ** int(np.floor(np.log2(FP8MAX / np.abs(cw).max())))
    SR = 2.0 ** int(np.floor(np.log2(FP8MAX / np.abs(rel_weight).max())))

    big_chunks, small_chunks = [], []
    for c in range(NCORES):
        lo, hi = c * score, (c + 1) * score
        big_chunks.append(_pack_core(scope, seg, lo, hi, isbig))
        small_chunks.append(_pack_core(scope, seg, lo, hi, ~isbig))
    nb = max(len(ch) for ch in big_chunks)
    ns = max(len(ch) for ch in small_chunks)
    nb = (nb + SB - 1) // SB * SB
    ns = max(SB, (ns + SB - 1) // SB * SB)

    # shared consts. cwrel = [cw.T | rel.T] stacked per d-block.
    W6 = np.concatenate([cw, rel_weight], axis=0)          # [106, 690]
    # small: d = b*115 + p, b in 0..5
    cws = np.ascontiguousarray(
        W6.T.reshape(DB, DP, W2).transpose(1, 0, 2)
    ).reshape(DP, DB * W2).astype(bf16)
    # big (DoubleRow): d = blk*230 + j*115 + p, scaled
    W6s = np.concatenate([cw * SC, rel_weight * SR], axis=0)
    cwb = np.ascontiguousarray(
        W6s.T.reshape(3, 2, DP, W2).transpose(2, 0, 1, 3)
    ).reshape(DP, 3 * 2 * W2).astype(f8e4)
    io53 = np.ascontiguousarray(
        np.broadcast_to(np.arange(NCLS, dtype=np.float32), (CHUNK, NCLS))
    ).astype(bf16)
    io32 = np.ascontiguousarray(
        np.broadcast_to(np.arange(2 * BSLOT, dtype=np.float32),
                        (CHUNK, 2 * BSLOT))).astype(bf16)
    scv = np.full((CHUNK, 1), 1.0 / SC, np.float32)

    sched = _schedule(nb, ns)
    # global chunk k = pos*SB + u maps to section-local chunk lsb*SB + u
    gofA = np.zeros(nb // SB, np.int64)
    gofB = np.zeros(ns // SB, np.int64)
    for pos, (sec, lsb) in enumerate(sched):
        (gofA if sec == "A" else gofB)[lsb] = pos

    in_maps = []
    frag2bag = []
    for c in range(NCORES):
        # section-local sentence index arrays (for x packing)
        idxA = np.full(nb * CHUNK, -1, np.int64)
        idxB = np.full(ns * CHUNK, -1, np.int64)
        relseg = np.full((nb + ns) * CHUNK, PAD_SLOT, np.float32)
        qp = np.zeros((nb + ns) * CHUNK, np.float32)
        f2b = np.full((nb + ns, BSLOT), -1, np.int64)
        for sec, chs, idxL, gof in (("A", big_chunks[c], idxA, gofA),
                                    ("B", small_chunks[c], idxB, gofB)):
            for kk, ch in enumerate(chs):
                kglob = gof[kk // SB] * SB + kk % SB
                p = kk * CHUNK
                pg = kglob * CHUNK
                for j, (b, s, take) in enumerate(ch):
                    idxL[p:p + take] = np.arange(s, s + take)
                    relseg[pg:pg + take] = j + BSLOT * (kglob % 2)
                    f2b[kglob, j] = b
                    p += take
                    pg += take
                v = idxL[kk * CHUNK:(kk + 1) * CHUNK] >= 0
                qp[kglob * CHUNK:(kglob + 1) * CHUNK][v] = (
                    q[idxL[kk * CHUNK:(kk + 1) * CHUNK][v]].astype(np.float32))
        # big section x: [S_b, 690] -> [115, 3, S_b*2] DoubleRow interleave
        xsb = np.zeros((nb * CHUNK, DIM), np.float32)
        vb = idxA >= 0
        xsb[vb] = x[idxA[vb]]
        xtb = np.ascontiguousarray(
            xsb.reshape(nb, CHUNK, 3, 2, DP).transpose(4, 2, 0, 3, 1)
        ).reshape(DP, 3, nb * 2 * CHUNK).astype(f8e4)
        # small section x: [S_s, 690] -> [115, 6, S_s]
        xss = np.zeros((ns * CHUNK, DIM), np.float32)
        vs = idxB >= 0
        xss[vs] = x[idxB[vs]]
        xts = np.ascontiguousarray(
            xss.reshape(ns * CHUNK, DB, DP).transpose(2, 1, 0)).astype(f8e3)
        in_maps.append({
            "xtb": xtb,
            "xts": xts,
            "cwb": cwb,
            "cws": cws,
            "qv": np.ascontiguousarray(qp.reshape(nb + ns, CHUNK).T),
            "rs": np.ascontiguousarray(relseg.reshape(nb + ns, CHUNK).T),
            "io53": io53,
            "io32": io32,
            "sc": scv,
        })
        frag2bag.append(f2b)
    return in_maps, frag2bag, nb, ns, nbags, bias, SR


def _assemble(tables, frag2bag, nb, ns, nbags, bias, SR):
    num = np.zeros((nbags, NCLS))
    den = np.zeros(nbags)
    # fragment (k, j): table row (k//8)*32 + slot, col block ((k//2)%4)*54
    # where slot = j + 16*(k%2). Big-section numerators carry factor SR.
    ks = np.arange(nb + ns)[:, None]
    js = np.arange(BSLOT)[None, :]
    slot = js + BSLOT * (ks % 2)
    rows = ((ks // 8) * 32 + slot).ravel()
    cols = (((ks // 2) % 4) * (NCLS + 1) + np.zeros_like(slot)).ravel()
    isbigk = np.zeros(nb + ns, bool)
    for pos, (sec, lsb) in enumerate(_schedule(nb, ns)):
        if sec == "A":
            isbigk[pos * SB:(pos + 1) * SB] = True
    nsc = np.where(isbigk[:, None], 1.0 / SR, 1.0).repeat(BSLOT, 1).ravel()
    for c in range(NCORES):
        table = np.asarray(tables[c], dtype=np.float32)
        fb = frag2bag[c].ravel()
        v = fb >= 0
        r = rows[v]
        c0 = cols[v]
        fbv = fb[v]
        sc = nsc[v]
        for k in range(NCLS):
            num[:, k] += np.bincount(fbv, table[r, c0 + k] * sc,
                                     minlength=nbags)
        den += np.bincount(fbv, table[r, c0 + NCLS], minlength=nbags)
    return (num / den[:, None] + bias[None, :]).astype(np.float32)


def kernel(x, rel_weight, att_weight, bias, attention_query, scope):
    from concourse.bass_utils import run_bass_kernel_spmd

    in_maps, frag2bag, nb, ns, nbags, b, SR = _prepare(
        x, rel_weight, att_weight, bias, attention_query, scope)
    key = (nb, ns)
    if key not in _cache:
        _cache[key] = _build_module(nb, ns)
    nc = _cache[key]
    res = run_bass_kernel_spmd(nc, in_maps, list(range(NCORES)))
    tables = [res.results[c]["tab"] for c in range(NCORES)]
    return _assemble(tables, frag2bag, nb, ns, nbags, b, SR)
